# revision 29
# baseline (speedup 1.0000x reference)
"""GraphSAGE (2-layer, mean aggr) on 8 trn2 NeuronCores.

Strategy (graph/data parallel, per sharding hint):
 - Nodes sharded by range: core c owns dst nodes [c*6250, (c+1)*6250).
 - Host pre-sorts edges by (core, dst-tile, src-half) and builds int16
   gather indices (x split in two 25000-row halves since dma_gather idxs
   are int16).
 - Gathers are issued round-robin across 4 SWDGE queues so descriptor
   generation runs on all four GpSimd Q7 core pairs concurrently.
 - Segment-mean via one-hot matmuls on PE: S[e,d] = (iota==ld[e]) built in
   one batched DVE scalar_tensor_tensor per (tile, half) group; the 1/cnt
   normalization is applied on the PSUM->SBUF copy with a per-column
   invrow multiply.
 - Layer 1 (kernel A): gather x[src] (bf16 512B rows), one-hot matmuls
   accumulate sumT feat-major in PSUM; dense W1_l/W1_r quadrant matmuls +
   fused bias+ReLU on ACT produce hT per tile; y2T = W2_l.T @ hT and
   out2dT = W2_r.T @ hT + b2 are written feat-major.
 - Host transposes y2T -> y2 node-major and all-gathers across cores
   (layer boundary).
 - Layer 2 (kernel B): gather y2[src] (bf16 256B), same one-hot
   segment-sum, out = sum2T*invrow + out2dT, written feat-major; host
   transposes back.
"""

import numpy as np
import ml_dtypes

import concourse.bacc as bacc
import concourse.mybir as mybir
import concourse.tile as tile
from concourse.bass_utils import run_bass_kernel_spmd


def _timed_run(nc, in_maps, iters=12):
    """Mirror bass2jax.run_bass_via_pjrt's multi-core path, but keep the
    compiled executable, pre-place inputs on the device mesh, and pipeline
    `iters` back-to-back executions to estimate per-launch device time."""
    import time
    import jax
    import concourse.mybir as mb
    from concourse import bass2jax
    from jax.experimental.shard_map import shard_map
    from jax.sharding import Mesh, PartitionSpec, NamedSharding

    bass2jax.install_neuronx_cc_hook()
    n_cores = len(in_maps)
    partition_name = (nc.partition_id_tensor.name
                      if nc.partition_id_tensor else None)
    in_names, out_names, out_avals, zero_outs = [], [], [], []
    for alloc in nc.m.functions[0].allocations:
        if not isinstance(alloc, mb.MemoryLocationSet):
            continue
        name = alloc.memorylocations[0].name
        if alloc.kind == "ExternalInput":
            if name != partition_name:
                in_names.append(name)
        elif alloc.kind == "ExternalOutput":
            shape = tuple(alloc.tensor_shape)
            dtype = mb.dt.np(alloc.dtype)
            out_names.append(name)
            out_avals.append(jax.core.ShapedArray(shape, dtype))
            zero_outs.append(np.zeros(shape, dtype))
    n_params = len(in_names)
    n_outs = len(out_avals)
    in_names.extend(out_names)
    if partition_name is not None:
        in_names.append(partition_name)
    donate = tuple(range(n_params, n_params + n_outs))

    def _body(*args):
        operands = list(args)
        if partition_name is not None:
            operands.append(bass2jax.partition_id_tensor())
        outs = bass2jax._bass_exec_p.bind(
            *operands, out_avals=tuple(out_avals), in_names=tuple(in_names),
            out_names=tuple(out_names), lowering_input_output_aliases=(),
            sim_require_finite=True, sim_require_nnan=True, nc=nc)
        return tuple(outs)

    devices = jax.devices()[:n_cores]
    mesh = Mesh(np.asarray(devices), ("core",))
    in_specs = (PartitionSpec("core"),) * (n_params + n_outs)
    out_specs = (PartitionSpec("core"),) * len(out_names)
    sharded = jax.jit(
        shard_map(_body, mesh=mesh, in_specs=in_specs,
                  out_specs=out_specs, check_rep=False),
        donate_argnums=donate, keep_unused=True)
    sh = NamedSharding(mesh, PartitionSpec("core"))
    per_core = [[np.asarray(m[name]) for name in in_names[:n_params]]
                for m in in_maps]
    concat_in = [np.concatenate([per_core[c][i] for c in range(n_cores)], axis=0)
                 for i in range(n_params)]
    dev_in = [jax.device_put(a, sh) for a in concat_in]
    czs = [np.zeros((n_cores * z.shape[0], *z.shape[1:]), z.dtype)
           for z in zero_outs]
    # one warm-up (also produces the returned results)
    zo = [jax.device_put(z, sh) for z in czs]
    out_arrs = sharded(*dev_in, *zo)
    jax.block_until_ready(out_arrs)
    results = [
        {name: np.asarray(out_arrs[i]).reshape(n_cores, *out_avals[i].shape)[c]
         for i, name in enumerate(out_names)}
        for c in range(n_cores)]
    # marginal cost per extra launch: time batches of n1 and n2 pipelined
    # launches; slope strips the fixed dispatch/RPC overhead.
    n1, n2 = 3, iters + 3
    def batch(n):
        zsets = [[jax.device_put(z, sh) for z in czs] for _ in range(n)]
        jax.block_until_ready(zsets)
        t0 = time.perf_counter()
        outs = [sharded(*dev_in, *zsets[i]) for i in range(n)]
        jax.block_until_ready(outs)
        return time.perf_counter() - t0
    t1 = batch(n1)
    t2 = batch(n2)
    slope = (t2 - t1) / (n2 - n1)
    print(f"    batch{n1}={t1*1e3:.1f}ms batch{n2}={t2*1e3:.1f}ms "
          f"slope={slope*1e6:.0f}us/launch", flush=True)
    return results, slope * 1e9

def _try_ntff_shim():
    """Register the axon NTFF profiling hook if this container has it; lets
    run_bass_kernel_spmd(trace=True) return hardware exec_time_ns."""
    import sys
    import types
    if "antenv.axon_hooks" in sys.modules:
        return True
    try:
        sys.path.insert(0, "/root/.axon_site")
        from trn_agent_boot.trn_boot import _ntff_profile_via_ctypes
        hook = _ntff_profile_via_ctypes('/opt/axon/libaxon_pjrt.so')
        if hook is None:
            return False
        mod = types.ModuleType("antenv.axon_hooks")
        mod.get_axon_ntff_profile_hook = lambda: hook
        mod.set_axon_ntff_profile_hook = lambda h: None
        sys.modules["antenv.axon_hooks"] = mod
        return True
    except Exception:
        return False


BF16 = ml_dtypes.bfloat16

N_NODES = 50000
N_EDGES = 800000
D_IN, D_HID, D_OUT = 256, 256, 128
N_CORES = 8
NPC = N_NODES // N_CORES  # 6250
HALF = 25000              # x row-split so gather idx fits int16
M = 128                   # dst-tile width (PSUM free dim)
T = (NPC + M - 1) // M    # 49 dst tiles per core
GSPLIT = 8                # max 128-idx chunks per dma_gather instruction
NQ = 4                    # SWDGE queues (one Q7 core pair each)

LAST_EXEC_NS = {}


def _plan(edge_index):
    """Host-side graph preprocessing shared by both layers.

    Groups (tile, half) are visited with the half-order alternating per
    tile so adjacent groups with the same source half form segments whose
    gather calls can be merged (fewer dma_gather instructions)."""
    src = np.asarray(edge_index[0], dtype=np.int64)
    dst = np.asarray(edge_index[1], dtype=np.int64)
    E = src.shape[0]
    cnt = np.bincount(dst, minlength=N_NODES)
    inv = (1.0 / np.maximum(cnt, 1)).astype(np.float32)
    hh = src // HALF

    # --- dst-node permutation: balance (tile, half) bin counts across ---
    # cores so the SPMD max-over-cores chunk padding stays small. Nodes are
    # dealt round-by-round into the 8*T (core, tile) buckets; each round
    # matches the largest-degree nodes to the least-loaded buckets,
    # alternating which half's load is equalized.
    d0 = np.bincount(dst[hh == 0], minlength=N_NODES)
    d1 = np.bincount(dst[hh == 1], minlength=N_NODES)
    NB = N_CORES * T
    caps = np.full(NB, M, np.int64)
    caps.reshape(N_CORES, T)[:, T - 1] = NPC - (T - 1) * M
    s0 = np.zeros(NB)
    s1 = np.zeros(NB)
    bucket_of = np.full(N_NODES, -1, np.int64)
    nodes_by_deg = np.argsort(-(d0 + d1), kind="stable")
    left = caps.copy()
    pos0 = 0
    rnd = 0
    while pos0 < N_NODES:
        active = np.where(left > 0)[0]
        batch = nodes_by_deg[pos0:pos0 + len(active)]
        pos0 += len(batch)
        load = s0 if rnd % 2 == 0 else s1
        dd = d0 if rnd % 2 == 0 else d1
        border = active[np.argsort(load[active], kind="stable")]
        nodeorder = batch[np.argsort(-dd[batch], kind="stable")]
        border = border[:len(nodeorder)]
        bucket_of[nodeorder] = border
        s0[border] += d0[nodeorder]
        s1[border] += d1[nodeorder]
        left[border] -= 1
        rnd += 1
    # slot order within buckets -> new node ids; old_of_new = permutation
    border_sort = np.argsort(bucket_of * N_NODES + np.arange(N_NODES), kind="stable")
    # nodes grouped by bucket in bucket order
    bstart = np.concatenate([[0], np.cumsum(np.bincount(bucket_of, minlength=NB))])
    newid_of = np.empty(N_NODES, np.int64)
    old_of_new = np.empty(N_NODES, np.int64)
    for b in range(NB):
        c, t = b // T, b % T
        members = border_sort[bstart[b]:bstart[b + 1]]
        base = c * NPC + t * M
        ids = base + np.arange(len(members))
        newid_of[members] = ids
        old_of_new[ids] = members

    # visit order of (tile, half) groups; alternate halves to merge calls
    seq = [(t, h) for t in range(T) for h in ((0, 1) if t % 2 == 0 else (1, 0))]
    P = len(seq)  # 2T
    seqpos_of = np.zeros((T, 2), np.int64)
    for pos, (t, h) in enumerate(seq):
        seqpos_of[t, h] = pos

    nd = newid_of[dst]
    core = nd // NPC
    dloc = nd - core * NPC
    tt = dloc // M
    key = core * P + seqpos_of[tt, hh]
    order = np.argsort(key, kind="stable")
    skey = key[order]
    ssrc = src[order]
    sdloc = dloc[order]
    shh = hh[order]

    nbins = N_CORES * P
    bc = np.bincount(key, minlength=nbins).reshape(N_CORES, P)
    # uniform chunk count per group across cores -> one SPMD program
    Cseq = np.maximum(1, -(-bc.max(axis=0) // 128))  # [P]
    K = int(Cseq.sum())          # chunks per core
    TOT = K * 128                # padded idx slots per core

    seg_len = Cseq * 128
    slot_off = np.concatenate([[0], np.cumsum(seg_len)[:-1]])
    chunk_off = np.concatenate([[0], np.cumsum(Cseq)[:-1]])

    # within-group position of each sorted edge
    starts = np.concatenate([[0], np.cumsum(np.bincount(skey, minlength=nbins))[:-1]])
    within = np.arange(E) - starts[skey]

    pb = skey % P                        # group (seq position) per edge
    ch = chunk_off[pb] + within // 128   # chunk column
    p = within % 128                     # partition within chunk
    j = slot_off[pb] + within            # padded slot within its core's stream
    cidx = skey // P                     # core

    idx_arr = np.zeros((N_CORES, 128, TOT // 16), np.int16)
    ld_arr = np.full((N_CORES, 128, K), -1.0, np.float32)

    idxval = (ssrc - shh * HALF).astype(np.int16)
    for r in range(8):
        idx_arr[cidx, (j % 16) + 16 * r, j // 16] = idxval
    ld_arr[cidx, p, ch] = (sdloc % M).astype(np.float32)

    # group metadata in visit order
    groups = []
    k0 = 0
    for pos, (t, h) in enumerate(seq):
        C = int(Cseq[pos])
        groups.append(dict(t=t, h=h, C=C, k0=k0))
        k0 += C
    # segments: runs of same h; each carries its groups' positions
    segs = []
    for pos, grp in enumerate(groups):
        if segs and groups[segs[-1]["pos"][-1]]["h"] == grp["h"]:
            segs[-1]["pos"].append(pos)
            segs[-1]["Cs"] += grp["C"]
        else:
            segs.append(dict(h=grp["h"], pos=[pos], Cs=grp["C"]))
    seg_of_pos = {}
    for si, sg in enumerate(segs):
        off = 0
        for pos in sg["pos"]:
            seg_of_pos[pos] = (si, off)
            off += groups[pos]["C"]

    # per-core inv broadcast down partitions: invrow[c, p, m] = inv of the
    # (permuted) node at new id c*NPC+m
    inv_new = inv[old_of_new]
    invrow = np.broadcast_to(
        inv_new.reshape(N_CORES, 1, NPC), (N_CORES, 128, NPC)).astype(BF16)

    iota = np.broadcast_to(np.arange(128, dtype=np.float32), (128, 128))
    return dict(
        groups=groups, segs=segs, seg_of_pos=seg_of_pos, K=K, TOT=TOT,
        idx=idx_arr, ld=ld_arr.astype(BF16), invrow=np.ascontiguousarray(invrow),
        iota=iota.astype(BF16), old_of_new=old_of_new,
    )


def _common_consts(nc, dt, cpool, K, TOT):
    """Declare + preload tensors shared by both kernels."""
    iota_d = nc.dram_tensor("iota", [128, 128], dt.bfloat16, kind="ExternalInput")
    ld_d = nc.dram_tensor("ld", [128, K], dt.bfloat16, kind="ExternalInput")
    invrow_d = nc.dram_tensor("invrow", [128, NPC], dt.bfloat16, kind="ExternalInput")
    idx_d = nc.dram_tensor("idx", [128, TOT // 16], dt.int16, kind="ExternalInput")
    iota_t = cpool.tile([128, 128], dt.bfloat16, tag="iota")
    ld_t = cpool.tile([128, K], dt.bfloat16, tag="ld")
    invrow_t = cpool.tile([128, NPC], dt.bfloat16, tag="invrow")
    idx_t = cpool.tile([128, TOT // 16], dt.int16, tag="idx")
    # idx gates the first gathers: two slices on the Sync HWDGE queue (the
    # first small so the first gather starts early); everything else goes
    # on the Scalar HWDGE queue so the two const streams load in parallel.
    ncols = TOT // 16
    c0 = min(256, ncols)
    nc.sync.dma_start(idx_t[:, 0:c0], idx_d[:, 0:c0])
    if c0 < ncols:
        nc.sync.dma_start(idx_t[:, c0:], idx_d[:, c0:])
    nc.scalar.dma_start(iota_t[:], iota_d[:])
    nc.scalar.dma_start(ld_t[:], ld_d[:])
    nc.scalar.dma_start(invrow_t[:], invrow_d[:])
    return iota_t, ld_t, invrow_t, idx_t


def _build_S(nc, dt, spool, iota_t, ld_t, k0, C):
    """S[p, c, m] = (ld[p, k0+c] == iota[m]) as bf16 0/1, one DVE op."""
    AL = mybir.AluOpType
    S = spool.tile([128, C, M], dt.bfloat16, tag="S")
    ld_bc = ld_t[:, k0:k0 + C].unsqueeze(2).broadcast_to([128, C, M])
    iota_bc = iota_t[:].unsqueeze(1).broadcast_to([128, C, M])
    nc.vector.scalar_tensor_tensor(
        S[:], ld_bc, 0.0, iota_bc, op0=AL.add, op1=AL.is_equal)
    return S


def _issue_seg_gathers(nc, gpool, idx_t, src_d, sg, D, ioff, qstate, tag):
    """Issue the merged gather calls for one segment; returns (tile, ioff).

    Queue assignment MUST be strict round-robin in issue order: the tile
    framework rotates SWDGE DMAs over NUM_SWDGE_GLOBAL_SEMS (8) semaphore
    lanes in the same order and assumes in-order completion per lane, so
    lane i%8 must always map to the same SWDGE queue (i%4). Call sizes are
    balanced within each segment so round-robin stays load-balanced too."""
    Cs = sg["Cs"]
    gseg = gpool.tile([128, Cs, D], mybir.dt.bfloat16, tag=tag)
    n_calls = -(-Cs // GSPLIT)
    base, rem = divmod(Cs, n_calls)
    off = 0
    h = sg["h"]
    for ci in range(n_calls):
        cc = base + (1 if ci < rem else 0)
        ni = cc * 128
        q = qstate[0] % NQ
        qstate[0] += 1
        nc.gpsimd.dma_gather(
            gseg[:, off:off + cc, :], src_d[h * HALF:(h + 1) * HALF, :],
            idx_t[:, ioff:ioff + ni // 16], ni, ni, D,
            queue_num=q)
        off += cc
        ioff += ni // 16
    return gseg, ioff


def _build_A(plan):
    dt = mybir.dt
    groups, segs, seg_of_pos = plan["groups"], plan["segs"], plan["seg_of_pos"]
    K, TOT = plan["K"], plan["TOT"]
    nc = bacc.Bacc("TRN2", target_bir_lowering=False, debug=False,
                   num_devices=N_CORES, num_swdge_queues=NQ,
                   dynamic_dma_scratch_size=32768)
    x_nm = nc.dram_tensor("x_nm", [N_NODES, D_IN], dt.bfloat16, kind="ExternalInput")
    xT = nc.dram_tensor("xT", [D_IN, NPC], dt.bfloat16, kind="ExternalInput")
    w1l_d = nc.dram_tensor("w1l", [128, 2 * D_HID], dt.bfloat16, kind="ExternalInput")
    w1r_d = nc.dram_tensor("w1r", [128, 2 * D_HID], dt.bfloat16, kind="ExternalInput")
    w2l_d = nc.dram_tensor("w2l", [128, 2 * D_OUT], dt.bfloat16, kind="ExternalInput")
    w2r_d = nc.dram_tensor("w2r", [128, 2 * D_OUT], dt.bfloat16, kind="ExternalInput")
    b1_d = nc.dram_tensor("b1", [128, 2], dt.float32, kind="ExternalInput")
    b2_d = nc.dram_tensor("b2", [128, 1], dt.float32, kind="ExternalInput")
    y2T_o = nc.dram_tensor("y2T", [128, NPC], dt.bfloat16, kind="ExternalOutput")
    od_o = nc.dram_tensor("od", [128, NPC], dt.bfloat16, kind="ExternalOutput")

    AF = mybir.ActivationFunctionType
    AL = mybir.AluOpType
    qstate = [0]  # gather issue counter (strict round-robin queues)
    with tile.TileContext(nc) as tc:
        with (
            tc.tile_pool(name="const", bufs=1) as cpool,
            tc.tile_pool(name="gath", bufs=4) as gpool,
            tc.tile_pool(name="sone", bufs=4) as spool,
            tc.tile_pool(name="mm", bufs=2) as mpool,
            tc.tile_pool(name="yy", bufs=3) as ypool,
            tc.tile_pool(name="psA", bufs=2, space="PSUM") as ppA,
            tc.tile_pool(name="psE", bufs=1, space="PSUM") as ppE,
        ):
            iota_t, ld_t, invrow_t, idx_t = _common_consts(nc, dt, cpool, K, TOT)
            w1l_t = cpool.tile([128, 2 * D_HID], dt.bfloat16, tag="w1l")
            w1r_t = cpool.tile([128, 2 * D_HID], dt.bfloat16, tag="w1r")
            w2l_t = cpool.tile([128, 2 * D_OUT], dt.bfloat16, tag="w2l")
            w2r_t = cpool.tile([128, 2 * D_OUT], dt.bfloat16, tag="w2r")
            b1_t = cpool.tile([128, 2], dt.float32, tag="b1")
            b2_t = cpool.tile([128, 1], dt.float32, tag="b2")
            nc.scalar.dma_start(w1l_t[:], w1l_d[:])
            nc.scalar.dma_start(w1r_t[:], w1r_d[:])
            nc.scalar.dma_start(w2l_t[:], w2l_d[:])
            nc.scalar.dma_start(w2r_t[:], w2r_d[:])
            nc.scalar.dma_start(b1_t[:], b1_d[:])
            nc.scalar.dma_start(b2_t[:], b2_d[:])

            ioff = 0
            seg_tiles = {}
            pa = pb = None
            ci = nch = 0
            for pos, grp in enumerate(groups):
                t, C, k0 = grp["t"], grp["C"], grp["k0"]
                n0 = t * M
                Mt = min(M, NPC - n0)
                si, soff = seg_of_pos[pos]
                if soff == 0:
                    seg_tiles[si], ioff = _issue_seg_gathers(
                        nc, gpool, idx_t, x_nm, segs[si], D_IN, ioff, qstate,
                        tag="gseg")
                g = seg_tiles[si]
                if ci == 0:
                    pa = ppA.tile([128, M], dt.float32, tag="pa")
                    pb = ppA.tile([128, M], dt.float32, tag="pb")
                    nch = C + groups[pos + 1]["C"]
                S = _build_S(nc, dt, spool, iota_t, ld_t, k0, C)
                for jj in range(C):
                    nc.tensor.matmul(pa[:, :Mt], g[:, soff + jj, 0:128],
                                     S[:, jj, :Mt],
                                     start=(ci == 0), stop=(ci == nch - 1))
                    nc.tensor.matmul(pb[:, :Mt], g[:, soff + jj, 128:256],
                                     S[:, jj, :Mt],
                                     start=(ci == 0), stop=(ci == nch - 1))
                    ci += 1
                if ci < nch:
                    continue  # first group of the tile done; epilogue after 2nd
                ci = 0
                # mean tiles (feat-major), normalized by invrow on the copy
                m1a = mpool.tile([128, M], dt.bfloat16, tag="m1a")
                m1b = mpool.tile([128, M], dt.bfloat16, tag="m1b")
                nc.vector.scalar_tensor_tensor(
                    m1a[:, :Mt], pa[:, :Mt], 0.0, invrow_t[:, n0:n0 + Mt],
                    op0=AL.add, op1=AL.mult)
                nc.vector.scalar_tensor_tensor(
                    m1b[:, :Mt], pb[:, :Mt], 0.0, invrow_t[:, n0:n0 + Mt],
                    op0=AL.add, op1=AL.mult)
                xta = mpool.tile([128, M], dt.bfloat16, tag="xta")
                xtb = mpool.tile([128, M], dt.bfloat16, tag="xtb")
                nc.scalar.dma_start(xta[:, :Mt], xT[0:128, n0:n0 + Mt])
                nc.scalar.dma_start(xtb[:, :Mt], xT[128:256, n0:n0 + Mt])
                pha = ppE.tile([128, M], dt.float32, tag="pha")
                phb = ppE.tile([128, M], dt.float32, tag="phb")
                # hT[hh] = W1_l[kh,hh].T @ mean1T[kh] + W1_r[kh,hh].T @ xT[kh]
                nc.tensor.matmul(pha[:, :Mt], w1l_t[:, 0:128], m1a[:, :Mt], start=True, stop=False)
                nc.tensor.matmul(pha[:, :Mt], w1l_t[:, 256:384], m1b[:, :Mt], start=False, stop=False)
                nc.tensor.matmul(pha[:, :Mt], w1r_t[:, 0:128], xta[:, :Mt], start=False, stop=False)
                nc.tensor.matmul(pha[:, :Mt], w1r_t[:, 256:384], xtb[:, :Mt], start=False, stop=True)
                nc.tensor.matmul(phb[:, :Mt], w1l_t[:, 128:256], m1a[:, :Mt], start=True, stop=False)
                nc.tensor.matmul(phb[:, :Mt], w1l_t[:, 384:512], m1b[:, :Mt], start=False, stop=False)
                nc.tensor.matmul(phb[:, :Mt], w1r_t[:, 128:256], xta[:, :Mt], start=False, stop=False)
                nc.tensor.matmul(phb[:, :Mt], w1r_t[:, 384:512], xtb[:, :Mt], start=False, stop=True)
                hta = mpool.tile([128, M], dt.bfloat16, tag="hta")
                htb = mpool.tile([128, M], dt.bfloat16, tag="htb")
                nc.scalar.activation(hta[:, :Mt], pha[:, :Mt], AF.Relu,
                                     bias=b1_t[:, 0:1])
                nc.scalar.activation(htb[:, :Mt], phb[:, :Mt], AF.Relu,
                                     bias=b1_t[:, 1:2])
                # y2T = W2_l.T @ hT (feat-major out; host transposes)
                py2 = ppE.tile([128, M], dt.float32, tag="py2")
                nc.tensor.matmul(py2[:, :Mt], w2l_t[:, 0:128],
                                 hta[:, :Mt], start=True, stop=False)
                nc.tensor.matmul(py2[:, :Mt], w2l_t[:, 128:256],
                                 htb[:, :Mt], start=False, stop=True)
                y2st = ypool.tile([128, M], dt.bfloat16, tag="y2st")
                nc.vector.tensor_copy(y2st[:, :Mt], py2[:, :Mt])
                nc.sync.dma_start(y2T_o[:, n0:n0 + Mt], y2st[:, :Mt])
                # out2dT = W2_r.T @ hT + b2 (layer-2 dense term, done here)
                pd = ppE.tile([128, M], dt.float32, tag="pd")
                nc.tensor.matmul(pd[:, :Mt], w2r_t[:, 0:128],
                                 hta[:, :Mt], start=True, stop=False)
                nc.tensor.matmul(pd[:, :Mt], w2r_t[:, 128:256],
                                 htb[:, :Mt], start=False, stop=True)
                odst = ypool.tile([128, M], dt.bfloat16, tag="odst")
                nc.vector.tensor_scalar(odst[:, :Mt], pd[:, :Mt],
                                        b2_t[:, 0:1], None, op0=AL.add)
                nc.sync.dma_start(od_o[:, n0:n0 + Mt], odst[:, :Mt])
    nc.compile()
    return nc


def _build_B(plan):
    dt = mybir.dt
    groups, segs, seg_of_pos = plan["groups"], plan["segs"], plan["seg_of_pos"]
    K, TOT = plan["K"], plan["TOT"]
    nc = bacc.Bacc("TRN2", target_bir_lowering=False, debug=False,
                   num_devices=N_CORES, num_swdge_queues=NQ,
                   dynamic_dma_scratch_size=32768)
    y2f = nc.dram_tensor("y2f", [N_NODES, D_OUT], dt.bfloat16, kind="ExternalInput")
    od_d = nc.dram_tensor("od", [128, NPC], dt.bfloat16, kind="ExternalInput")
    outT = nc.dram_tensor("outT", [128, NPC], dt.bfloat16, kind="ExternalOutput")

    AL = mybir.AluOpType
    qstate = [0]  # gather issue counter (strict round-robin queues)
    with tile.TileContext(nc) as tc:
        with (
            tc.tile_pool(name="const", bufs=1) as cpool,
            tc.tile_pool(name="gath", bufs=4) as gpool,
            tc.tile_pool(name="sone", bufs=4) as spool,
            tc.tile_pool(name="oo", bufs=3) as opool,
            tc.tile_pool(name="ps", bufs=2, space="PSUM") as pp,
        ):
            iota_t, ld_t, invrow_t, idx_t = _common_consts(nc, dt, cpool, K, TOT)
            od_t = cpool.tile([128, NPC], dt.bfloat16, tag="od")
            nc.scalar.dma_start(od_t[:], od_d[:])

            ioff = 0
            seg_tiles = {}
            p2 = None
            ci = nch = 0
            for pos, grp in enumerate(groups):
                t, C, k0 = grp["t"], grp["C"], grp["k0"]
                n0 = t * M
                Mt = min(M, NPC - n0)
                si, soff = seg_of_pos[pos]
                if soff == 0:
                    seg_tiles[si], ioff = _issue_seg_gathers(
                        nc, gpool, idx_t, y2f, segs[si], D_OUT, ioff, qstate,
                        tag="gseg")
                g = seg_tiles[si]
                if ci == 0:
                    p2 = pp.tile([128, M], dt.float32, tag="p2")
                    nch = C + groups[pos + 1]["C"]
                S = _build_S(nc, dt, spool, iota_t, ld_t, k0, C)
                for jj in range(C):
                    nc.tensor.matmul(p2[:, :Mt], g[:, soff + jj, :],
                                     S[:, jj, :Mt],
                                     start=(ci == 0), stop=(ci == nch - 1))
                    ci += 1
                if ci < nch:
                    continue
                ci = 0
                # out = sum2T * invrow + out2dT
                tmp = opool.tile([128, M], dt.bfloat16, tag="tmp")
                outst = opool.tile([128, M], dt.bfloat16, tag="outst")
                nc.vector.scalar_tensor_tensor(
                    tmp[:, :Mt], p2[:, :Mt], 0.0, invrow_t[:, n0:n0 + Mt],
                    op0=AL.add, op1=AL.mult)
                nc.vector.scalar_tensor_tensor(
                    outst[:, :Mt], tmp[:, :Mt], 0.0, od_t[:, n0:n0 + Mt],
                    op0=AL.add, op1=AL.add)
                nc.sync.dma_start(outT[:, n0:n0 + Mt], outst[:, :Mt])
    nc.compile()
    return nc


def _arrange_w(w):
    """[2K x N] -> [128, 2N]: out[k, kh*N + n] = w[kh*128 + k, n]"""
    K2, N = w.shape
    return np.concatenate([w[0:128, :], w[128:256, :]], axis=1)


def kernel(x, edge_index, W1_l, b1, W1_r, W2_l, b2, W2_r, _trace=False):
    x = np.asarray(x, dtype=np.float32)
    plan = _plan(edge_index)

    x_bf = x.astype(BF16)
    w1l_a = _arrange_w(np.asarray(W1_l, np.float32)).astype(BF16)
    w1r_a = _arrange_w(np.asarray(W1_r, np.float32)).astype(BF16)
    w2l_a = _arrange_w(np.asarray(W2_l, np.float32)).astype(BF16)
    w2r_a = _arrange_w(np.asarray(W2_r, np.float32)).astype(BF16)
    b1_a = np.asarray(b1, np.float32).reshape(2, 128).T.copy()
    b2_a = np.asarray(b2, np.float32).reshape(1, 128).T.copy()

    # ---- kernel A ----
    ncA = _build_A(plan)
    oon = plan["old_of_new"]
    in_maps_A = []
    for c in range(N_CORES):
        r0 = c * NPC
        in_maps_A.append({
            "x_nm": x_bf,
            "xT": np.ascontiguousarray(x_bf[oon[r0:r0 + NPC], :].T),
            "w1l": w1l_a, "w1r": w1r_a, "w2l": w2l_a, "w2r": w2r_a,
            "b1": b1_a, "b2": b2_a,
            "iota": plan["iota"], "ld": plan["ld"][c],
            "invrow": plan["invrow"][c], "idx": plan["idx"][c],
        })
    if _trace and not _try_ntff_shim():
        outsA, tA = _timed_run(ncA, in_maps_A)
        LAST_EXEC_NS["A"] = tA
    else:
        resA = run_bass_kernel_spmd(ncA, in_maps_A, list(range(N_CORES)),
                                    trace=_trace)
        LAST_EXEC_NS["A"] = resA.exec_time_ns
        outsA = resA.results

    # host layer boundary: transpose y2T -> node-major (un-permuting back
    # to original node ids) and all-gather
    y2f = np.empty((N_NODES, D_OUT), BF16)
    for c in range(N_CORES):
        y2f[oon[c * NPC:(c + 1) * NPC], :] = outsA[c]["y2T"].T

    # ---- kernel B ----
    ncB = _build_B(plan)
    in_maps_B = []
    for c in range(N_CORES):
        in_maps_B.append({
            "y2f": y2f,
            "od": outsA[c]["od"],
            "iota": plan["iota"], "ld": plan["ld"][c],
            "invrow": plan["invrow"][c], "idx": plan["idx"][c],
        })
    if _trace and not _try_ntff_shim():
        outsB, tB = _timed_run(ncB, in_maps_B)
        LAST_EXEC_NS["B"] = tB
    else:
        resB = run_bass_kernel_spmd(ncB, in_maps_B, list(range(N_CORES)),
                                    trace=_trace)
        LAST_EXEC_NS["B"] = resB.exec_time_ns
        outsB = resB.results

    out = np.empty((N_NODES, D_OUT), np.float32)
    for c in range(N_CORES):
        out[oon[c * NPC:(c + 1) * NPC], :] = outsB[c]["outT"].T.astype(np.float32)
    return out
